# revision 32
# baseline (speedup 1.0000x reference)
"""DeepEMD Trainium2 kernel: batched 49x49 entropic-OT (Sinkhorn) similarity.

Final strategy (8 NeuronCores, data-parallel over batch; 128 batches/core):
- Host prepacks per (group of 32 batches) slabs A = [Q | 1 | P | pad]
  (128ch x 100, fp16). The ones column sits at col 49 so every gram row
  phase B needs lands in contiguous output rows 0:50; the pad col keeps
  every block 4B-aligned so DVE ops hit the 2x packed mode.
- Phase A: 4 group loads in 2.3us quarter-granules (keeps PE idle gaps
  under the ~3.4us HAM re-throttle window). Per batch, one PSUM
  accumulation group of 8 matmuls: 4 chunks x (W^T M1 + W^T M2) with
  W = [Q|1|P] widened to 128 cols (FWL), M1 = [Q|1|P], M2 = P^2.
  P^2 is squared on DVE (2x mode), one group ahead of use. Output rows
  0:49 = [QtQ (diag dq) | . | QtP]; row 49 = [sq | C | sp | dp].
- Gram flatten (gram-major -> batch-major flatG[128, 50*148]) via a DRAM
  bounce: hop1 copies each 16-batch gs block to DRAM verbatim (50 fat
  4736B descriptors, all on the gpsimd queue so hop2 waits are FIFO-local
  or monotone), hop2 reads it back batch-major in 8-batch ops (296B
  descriptors, round-robin over all 3 DMA queues). DMA queues generate
  descriptors at ~10-15ns each, so descriptor count is the binding
  resource: the row-0:50 trim halves it vs a full-gram flatten (6400 vs
  12672) and the bounce replaces 128 per-batch SBUF->SBUF ops (~1.7us
  per-op overhead each) with 24 ops.
- Phase B: all big tensors fp16 in a [49, 50]-padded layout (2x DVE).
  Batched rsqrt fixups (one ACT round trip); sim = (qtp - sq x sp/C) *
  (inq x inp); K = exp(20*sim) directly (the e^{-1/eps} rescale cancels
  in the final normalization); Kt via ACT reading sim transposed,
  overlapped with the w1/w2 reduces on DVE. Linear Gauss-Seidel
  Sinkhorn, ITERS=4 with the last vs-update skipped (flow mass then =
  sum(w1), so logits normalize by 1/s1). w1/w2 are scaled by 256 so the
  fp16 iterates stay out of the subnormal range (cancels in 1/s1).
  KS=K*sim on gpsimd off the critical path.
Measured: ~152-155us (baseline v2: ~193-196us), rel err 4.96e-3.
"""

import os
import sys

import numpy as np

sys.path.insert(0, "/opt/trn_rl_repo")

import concourse.bass as bass
import concourse.bacc as bacc
import concourse.mybir as mybir
from concourse import tile
from concourse import masks
from concourse.bass_utils import run_bass_kernel_spmd

B_FULL, C, HW = 1024, 512, 49
NCORE = 8
BS = B_FULL // NCORE  # 128 batches per core
NCH = C // 128  # 4 chunks of 128 channels
AC = 2 * HW + 1  # 99 used slab cols per (batch, chunk): [Q | 1 | P]
AC2 = 100  # slab block pitch (pad col keeps every block 4B-aligned)
GRP = 32  # batches per load group
NGRP = BS // GRP
FB = 16  # batches per gs block / staged block
PAD = 50  # padded inner dim (4B-aligned fp16 rows)
FW = HW * PAD  # 2450
ITERS = 4
HALF = True  # skip last vs-update; normalize by s1
EXPB = 1.0 / 0.05  # K = exp(sim/eps) (the e^{-1/eps} factor cancels)
SIMPAD = -600.0
TEMP = 12.5 / HW
CPG = 2  # grams per PSUM bank / per copy (2 x 148 f32 fits a 2KB bank)
NBLK = BS // FB  # 8 staged blocks
H2B = 8  # batches per hop2 op
GW = 50  # gram output rows kept (Q rows 0:49 + ones row 49)
PITCH = 148  # per-batch gram row pitch: [99 gram cols | 49 dp cols]
WSCL = 256.0  # scale w1/w2 so the fp16 Sinkhorn iterates stay normal
#   (u ~ 2e-6 unscaled is deep fp16-subnormal; scaling both marginals by
#   the same factor leaves v unchanged, scales u and the flow mass by
#   WSCL, and cancels exactly in the final 1/s1 normalization)

f32 = mybir.dt.float32
f16 = mybir.dt.float16
Alu = mybir.AluOpType
Act = mybir.ActivationFunctionType
AxX = mybir.AxisListType.X

SLAB = GRP * NCH * AC2  # 12800 cols per group slab
SQW = GRP * NCH * PAD  # 6400 cols per squared-P slab (50-pitch, aligned)


def build_nc(debug=False):
    nc = bacc.Bacc(None, target_bir_lowering=False, debug=debug)
    aug = nc.declare_dram_parameter("aug", [NGRP, 128, SLAB], f16, isOutput=False)
    outp = nc.declare_dram_parameter("out", [BS, 1], f32, isOutput=True)
    dbg_names = os.environ.get("KDBG", "")
    dbgt = {}
    for nm in [x for x in dbg_names.split(",") if x]:
        dbgt[nm] = nc.declare_dram_parameter(
            f"dbg_{nm}", [BS, GW * PITCH if nm == "flatG" else FW], f16, isOutput=True
        )

    with tile.TileContext(nc) as tc:
        with (
            tc.tile_pool(name="big", bufs=1) as big,
            tc.tile_pool(name="stage", bufs=3) as stg,
            tc.tile_pool(name="sqst", bufs=4) as sqs,
            tc.tile_pool(name="gblk", bufs=4) as gcp,
            tc.tile_pool(name="small", bufs=1) as sml,
            tc.tile_pool(name="psum", bufs=7, space="PSUM") as pp,
            tc.tile_pool(name="dstg", bufs=1, space="DRAM") as dsp,
        ):
            flatG = big.tile([BS, GW * PITCH], f16, tag="flatG", name="flatG")

            simb = big.tile([BS, FW], f16, tag="sim", name="sim")
            Kb = big.tile([BS, FW], f16, tag="K", name="K")
            Ktb = big.tile([BS, FW], f16, tag="Kt", name="Kt")
            tb = big.tile([BS, FW], f16, tag="tb", name="tb")
            KS = big.tile([BS, FW], f16, tag="KS", name="KS")
            # pad prep: sim pads -> exp 0; Kt pad col stays 0
            nc.vector.memset(simb[:], SIMPAD)
            nc.vector.memset(Ktb[:], 0.0)

            def s_t(tag, w=HW, dt=f32):
                return sml.tile([BS, w], dt, tag=tag, name=tag)

            us = s_t("us", PAD, f16)
            vs = s_t("vs", PAD, f16)
            nc.vector.memset(us[:], 0.0)
            nc.vector.memset(vs[:], 0.0)
            # warm the ACT Sqrt table at t=0 (scalar is idle all of phase A;
            # a lazy load would sit on the phase-B critical path)
            wrm0 = s_t("wrm0", 1)
            nc.vector.memset(wrm0[:], 1.0)
            nc.scalar.activation(wrm0[:], wrm0[:], Act.Sqrt)

            # staged DRAM blocks, one per 16-batch gs block (separate tiles
            # so hop1(k+1) never serializes behind hop2(k))
            staged = [
                dsp.tile([GW, FB * PITCH], f16, tag=f"stgd{k}", name=f"stgd{k}")
                for k in range(NBLK)
            ]

            # ---------------- Phase A: load + gram + bounce-flatten ----------
            ths = []
            sqbs = []
            for g in range(NGRP):
                th = stg.tile([128, SLAB], f16, tag="h", name="hg")
                ths.append(th)
                # split every load into quarters: grams gate on ~2.3us
                # load granules, keeping PE idle gaps under the ~3.4us HAM
                # re-throttle window (cold PE runs matmuls at half clock)
                nspl = 4
                sw = SLAB // nspl
                for ss in range(nspl):
                    nc.sync.dma_start(
                        th[:, ss * sw : (ss + 1) * sw],
                        aug[g, :, ss * sw : (ss + 1) * sw],
                    )

            def h1eng(blk):
                # hop1 on gpsimd: each hop2 that waits on a hop1 is then
                # either behind it in the same FIFO or on another engine with
                # a monotone wait -- no cross-engine DMA wait cycles. The
                # last two blocks go via sync (empty after the loads):
                # gpsimd's ring is still draining earlier hop2 descriptors,
                # which delays hop1 completion ~7us right at the tail.
                return nc.sync if blk >= NBLK - 2 else nc.gpsimd

            def h2eng(k):  # k = 2*blk + half
                if k >= 12:
                    return (nc.scalar, nc.sync)[k % 2]
                return (nc.gpsimd, nc.scalar, nc.sync)[k % 3]

            gs = None
            ps = None

            def emit_squares(g):
                # square the P columns of group g's slab on DVE (2x mode;
                # feeds the dp matmuls). Emitted one group ahead so the DVE
                # FIFO never makes the PE wait.
                th = ths[g]
                sqb = sqs.tile([128, SQW], f16, tag="sq", name="sqb")
                sqbs.append(sqb)
                thv = th[:].rearrange("c (n a) -> c n a", a=AC2)[:, :, HW + 1 : AC]
                sqv = sqb[:].rearrange("c (n a) -> c n a", a=PAD)[:, :, 0:HW]
                NB = GRP * NCH
                nsq = 4
                for hh in range(nsq):
                    nc.vector.tensor_mul(
                        sqv[:, hh * NB // nsq : (hh + 1) * NB // nsq, :],
                        thv[:, hh * NB // nsq : (hh + 1) * NB // nsq, :],
                        thv[:, hh * NB // nsq : (hh + 1) * NB // nsq, :],
                    )

            emit_squares(0)
            for g in range(NGRP):
                th = ths[g]
                for b in range(GRP):
                    if b == GRP // 2 and g + 1 < NGRP:
                        emit_squares(g + 1)
                    gb = g * GRP + b
                    blk, bb = gb // FB, gb % FB
                    if bb == 0:
                        gs = gcp.tile([GW, FB * PITCH], f16, tag="gs", name="gs")
                    cb = bb % CPG
                    if cb == 0:
                        ps = pp.tile([128, CPG * PITCH], f32, tag="gram", name="gram")
                    # start/stop must cover one partition range: widen all 4
                    # chunks or none (no widening only for the slab tail)
                    wid = 128 if b * (NCH * AC2) + (NCH - 1) * AC2 + 128 <= SLAB else AC
                    # one accumulation group for both column ranges: start
                    # clears has_written range-wide, so only the first MM may
                    # carry it and only the last carries stop
                    for j in range(NCH):
                        base = b * (NCH * AC2) + j * AC2
                        sqbase = b * (NCH * PAD) + j * PAD
                        nc.tensor.matmul(
                            ps[0:wid, cb * PITCH : cb * PITCH + AC],
                            th[:, base : base + wid],
                            th[:, base : base + AC],
                            start=(j == 0),
                            stop=False,
                        )
                        nc.tensor.matmul(
                            ps[0:wid, cb * PITCH + AC : (cb + 1) * PITCH],
                            th[:, base : base + wid],
                            sqbs[g][:, sqbase : sqbase + HW],
                            start=False,
                            stop=(j == NCH - 1),
                        )
                    if cb == CPG - 1:
                        nc.vector.tensor_copy(
                            gs[:, (bb - 1) * PITCH : (bb + 1) * PITCH], ps[0:GW, :]
                        )
                    if bb == FB - 1:
                        # hop1: gs block -> DRAM verbatim (50 fat descriptors)
                        h1eng(blk).dma_start(staged[blk][:, :], gs[:, :])
                        # hop2: DRAM -> batch-major flatG rows (2 ops of 8
                        # batches; 296B descriptors)
                        for half in range(FB // H2B):
                            b0 = half * H2B
                            sv = staged[blk][
                                :, b0 * PITCH : (b0 + H2B) * PITCH
                            ].copy()
                            sv.ap = mybir.VecI64Pair(
                                [[PITCH, H2B], [FB * PITCH, GW], [1, PITCH]]
                            )
                            gb0 = blk * FB + b0
                            h2eng(2 * blk + half).dma_start(
                                flatG[gb0 : gb0 + H2B, :], sv
                            )

            # ---------------- Phase B: fixups ----------------
            # flatG row layout per batch: rows i=0:49 are [QtQ(49) | c | QtP(49)
            # | x(49)] at pitch 148; row 49 is [sq(49) | C | sp(49) | dp(49)]
            R49 = (GW - 1) * PITCH

            def dview(col0, stride):
                v = flatG[:, col0 : col0 + 1].copy()
                v.ap = mybir.VecI64Pair([list(v.ap[0])] + [[stride, HW]])
                return v

            dq = dview(0, PITCH + 1)
            sq = flatG[:, R49 : R49 + HW]
            sp = flatG[:, R49 + HW + 1 : R49 + 2 * HW + 1]
            dp = flatG[:, R49 + AC : R49 + AC + HW]

            # batched rsqrt chain over [sq|sp], [dq|dp] (one pass, one
            # ACT round trip instead of two)
            d98 = s_t("d98", 2 * HW)
            s98 = s_t("s98", 2 * HW)
            t98, v98, iv98, n98 = (
                s_t("t98", 2 * HW),
                s_t("v98", 2 * HW),
                s_t("iv98", 2 * HW),
                s_t("n98", 2 * HW),
            )
            inv98 = s_t("inv98", 2 * HW, f16)
            with tc.high_priority():
                nc.vector.tensor_copy(d98[:, 0:HW], dq)
                nc.vector.tensor_copy(d98[:, HW : 2 * HW], dp)
                nc.vector.tensor_copy(s98[:, 0:HW], sq)
                nc.vector.tensor_copy(s98[:, HW : 2 * HW], sp)
                nc.vector.tensor_mul(t98[:], s98[:], s98[:])
                nc.vector.scalar_tensor_tensor(
                    v98[:], t98[:], -1.0 / C, d98[:], Alu.mult, Alu.add
                )
                nc.scalar.activation(t98[:], v98[:], Act.Sqrt)
                nc.vector.reciprocal(iv98[:], t98[:])
                nc.vector.tensor_mul(n98[:], iv98[:], iv98[:])
                nc.vector.tensor_mul(n98[:], n98[:], v98[:])
                nc.vector.tensor_scalar(n98[:], n98[:], -0.5, 1.5, Alu.mult, Alu.add)
                nc.vector.tensor_mul(inv98[:], iv98[:], n98[:])
            inq = inv98[:, 0:HW]
            inp_ = inv98[:, HW : 2 * HW]
            # preload the Exp table now (both Sqrt uses are done) so the
            # load hides under the sim-build DVE ops
            wrm = s_t("wrm", 1)
            nc.vector.memset(wrm[:], 1.0)
            nc.scalar.activation(wrm[:], wrm[:], Act.Exp)

            # sim = (qtp - sq x sp / C) * (inq x inp), in [49,50]-padded fp16
            G3 = flatG[:].rearrange("b (i k) -> b i k", k=PITCH)
            qtp3 = G3[:, 0:HW, PAD : PAD + HW]
            sim3 = simb[:].rearrange("b (q p) -> b q p", p=PAD)
            KS3 = KS[:].rearrange("b (q p) -> b q p", p=PAD)
            tb3 = tb[:].rearrange("b (q p) -> b q p", p=PAD)
            bq = inq.unsqueeze(2).broadcast_to([BS, HW, HW])
            bp = inp_.unsqueeze(1).broadcast_to([BS, HW, HW])
            bsq = sq.unsqueeze(2).broadcast_to([BS, HW, HW])
            bsp = sp.unsqueeze(1).broadcast_to([BS, HW, HW])
            # ssp2 = (-sq/C) x sp on gpsimd (flatG-only deps, starts the
            # moment flatG lands), concurrent with the DVE fixup chain
            s49 = s_t("s49", HW, f16)
            nc.vector.tensor_scalar_mul(s49[:], sq, -1.0 / C)
            bs49 = s49[:].unsqueeze(2).broadcast_to([BS, HW, HW])
            nc.gpsimd.tensor_mul(KS3[:, :, 0:HW], bs49, bsp)  # KS as scratch
            # nrm = inq x inp on DVE (needs the fixups); then centering and
            # the final product, both 2x. High priority: this is the
            # critical path into K-exp and the Sinkhorn.
            with tc.high_priority(offset=1000):
                nc.vector.tensor_mul(tb3[:, :, 0:HW], bq, bp)
                nc.vector.tensor_add(sim3[:, :, 0:HW], qtp3, KS3[:, :, 0:HW])
                nc.vector.tensor_mul(
                    sim3[:, :, 0:HW], sim3[:, :, 0:HW], tb3[:, :, 0:HW]
                )

            # K = exp(sim/eps); Kt via transposed read (ACT)
            nc.scalar.activation(Kb[:], simb[:], Act.Exp, scale=EXPB)
            simT = simb[:].rearrange("b (q p) -> b p q", p=PAD)[:, 0:HW, :]
            KtV = Ktb[:].rearrange("b (p q) -> b p q", q=PAD)[:, :, 0:HW]
            nc.scalar.activation(KtV, simT, Act.Exp, scale=EXPB)

            # w1/w2 reduces on DVE, overlapped with the ACT exps (w2r is
            # the slow strided one -- emitted last so it hides under K-exp)
            w1r, w2r = s_t("w1r"), s_t("w2r")
            w1f = s_t("w1f", HW, f16)
            w2f = s_t("w2f", HW, f16)
            s1s = s_t("s1s", 1)
            nc.vector.tensor_reduce(w1r[:], qtp3, axis=AxX, op=Alu.add)
            G3T = flatG[:].rearrange("b (i k) -> b k i", k=PITCH)
            qtpT = G3T[:, PAD : PAD + HW, 0:HW]  # [b, p, q(stride PITCH)]
            nc.vector.tensor_reduce(
                w2r[:, 0:25], qtpT[:, 0:25, :], axis=AxX, op=Alu.add
            )
            nc.vector.tensor_reduce(
                w2r[:, 25:HW], qtpT[:, 25:HW, :], axis=AxX, op=Alu.add
            )
            for wr, wf in ((w1r, w1f), (w2r, w2f)):
                nc.vector.tensor_scalar(wr[:], wr[:], WSCL / HW, 0.0, Alu.mult, Alu.max)
                nc.vector.tensor_scalar(wr[:], wr[:], 0.001 * WSCL, None, Alu.add)
                nc.vector.tensor_copy(wf[:], wr[:])
            nc.vector.tensor_reduce(s1s[:], w1r[:], axis=AxX, op=Alu.add)

            # KS = K * sim for the final logits (gpsimd, off critical path)
            nc.gpsimd.tensor_mul(KS[:], Kb[:], simb[:])

            # ---------------- Phase B: Sinkhorn ----------------
            K3 = Kb[:].rearrange("b (q p) -> b q p", p=PAD)
            Kt3 = Ktb[:].rearrange("b (p q) -> b p q", q=PAD)
            kv, rkv = s_t("kv"), s_t("rkv")
            bvs = vs[:].unsqueeze(1).broadcast_to([BS, HW, PAD])
            bus = us[:].unsqueeze(1).broadcast_to([BS, HW, PAD])
            for it in range(ITERS):
                # u-step: kv[q] = sum_p K[q,p] v[p]; u = w1 / kv
                if it == 0:
                    nc.vector.tensor_reduce(kv[:], K3, axis=AxX, op=Alu.add)
                else:
                    nc.vector.tensor_mul(tb3, K3, bvs)
                    nc.vector.tensor_reduce(kv[:], tb3, axis=AxX, op=Alu.add)
                nc.vector.reciprocal_approx_fast(rkv[:], kv[:])
                nc.vector.tensor_mul(us[:, 0:HW], w1f[:], rkv[:])
                if HALF and it == ITERS - 1:
                    break
                # v-step: ku[p] = sum_q K[q,p] u[q]; v = w2 / ku
                nc.vector.tensor_mul(tb3, Kt3, bus)
                nc.vector.tensor_reduce(kv[:], tb3, axis=AxX, op=Alu.add)
                nc.vector.reciprocal_approx_fast(rkv[:], kv[:])
                nc.vector.tensor_mul(vs[:, 0:HW], w2f[:], rkv[:])

            # ---------------- Phase B: logits ----------------
            lgr = s_t("lgr", 1)
            lgf = s_t("lgf", 1)
            nc.vector.tensor_mul(tb3, KS3, bvs)  # KS * vs
            nc.vector.tensor_reduce(kv[:], tb3, axis=AxX, op=Alu.add)
            nc.vector.tensor_mul(kv[:], kv[:], w1r[:])
            nc.vector.tensor_mul(kv[:], kv[:], rkv[:])
            nc.vector.tensor_reduce(lgr[:], kv[:], axis=AxX, op=Alu.add)
            nc.vector.reciprocal(rkv[:, 0:1], s1s[:])
            nc.vector.scalar_tensor_tensor(
                lgf[:], lgr[:], TEMP, rkv[:, 0:1], Alu.mult, Alu.mult
            )
            nc.sync.dma_start(outp[:, :], lgf[:])
            if "flatG" in dbgt:
                nc.sync.dma_start(dbgt["flatG"][:, :], flatG[:])
            if "sim" in dbgt:
                nc.sync.dma_start(dbgt["sim"][:, :], simb[:])
            if "K" in dbgt:
                nc.sync.dma_start(dbgt["K"][:, :], Kb[:])
            if "Kt" in dbgt:
                nc.sync.dma_start(dbgt["Kt"][:, :], Ktb[:])

    nc.compile()
    return nc


_NC = None


def _get_nc():
    global _NC
    if _NC is None:
        _NC = build_nc()
    return _NC


def _prep_in_maps(feature_map1, feature_map2):
    q = np.ascontiguousarray(np.asarray(feature_map1, dtype=np.float32)).reshape(
        B_FULL, C, HW
    )
    p = np.ascontiguousarray(np.asarray(feature_map2, dtype=np.float32)).reshape(
        B_FULL, C, HW
    )
    in_maps = []
    for i in range(NCORE):
        sl = slice(i * BS, (i + 1) * BS)
        a32 = np.empty((NGRP, 128, GRP, NCH, AC2), np.float32)
        a32[..., HW] = 1.0
        a32[..., AC2 - 1] = 0.0
        qc = q[sl].reshape(NGRP, GRP, NCH, 128, HW).transpose(0, 3, 1, 2, 4)
        pc = p[sl].reshape(NGRP, GRP, NCH, 128, HW).transpose(0, 3, 1, 2, 4)
        a32[..., 0:HW] = qc
        a32[..., HW + 1 : AC] = pc
        in_maps.append({"aug": a32.astype(np.float16).reshape(NGRP, 128, SLAB)})
    return in_maps


def run(feature_map1, feature_map2, trace=False):
    in_maps = _prep_in_maps(feature_map1, feature_map2)
    nc = _get_nc()
    res = run_bass_kernel_spmd(nc, in_maps, core_ids=list(range(NCORE)), trace=trace)
    out = np.concatenate(
        [np.asarray(res.results[i]["out"]).reshape(BS) for i in range(NCORE)]
    ).astype(np.float32)
    return out, res


def kernel(feature_map1, feature_map2):
    out, _ = run(feature_map1, feature_map2, trace=False)
    return out


# revision 34
# speedup vs baseline: 1.1152x; 1.1152x over previous
"""DeepEMD Trainium2 kernel: batched 49x49 entropic-OT (Sinkhorn) similarity.

Final strategy (8 NeuronCores, data-parallel over batch; 128 batches/core):
- Host prepacks per (group of 32 batches) slabs A = [Q | 1 | P | pad]
  (128ch x 100, fp16). The ones column sits at col 49 so every gram row
  phase B needs lands in contiguous output rows 0:50; the pad col keeps
  every block 4B-aligned so DVE ops hit the 2x packed mode.
- Phase A: 4 group loads in 2.3us quarter-granules (keeps PE idle gaps
  under the ~3.4us HAM re-throttle window). Per batch, one PSUM
  accumulation group of 8 matmuls: 4 chunks x (W^T M1 + W^T M2) with
  W = [Q|1|P] widened to 128 cols (FWL), M1 = [Q|1|P], M2 = P^2.
  P^2 is squared on DVE (2x mode), one group ahead of use. Output rows
  0:49 = [QtQ (diag dq) | . | QtP]; row 49 = [sq | C | sp | dp].
- Gram flatten (gram-major -> batch-major flatG[128, 50*148]) via a DRAM
  bounce: hop1 copies each 16-batch gs block to DRAM verbatim (50 fat
  4736B descriptors, all on the gpsimd queue so hop2 waits are FIFO-local
  or monotone), hop2 reads it back batch-major in 8-batch ops (296B
  descriptors, round-robin over all 3 DMA queues). DMA queues generate
  descriptors at ~10-15ns each, so descriptor count is the binding
  resource: the row-0:50 trim halves it vs a full-gram flatten (6400 vs
  12672) and the bounce replaces 128 per-batch SBUF->SBUF ops (~1.7us
  per-op overhead each) with 24 ops.
- Phase B: all big tensors fp16 in a [49, 50]-padded layout (2x DVE).
  Batched rsqrt fixups (one ACT round trip); sim = (qtp - sq x sp/C) *
  (inq x inp); K = exp(20*sim) directly (the e^{-1/eps} rescale cancels
  in the final normalization); Kt via ACT reading sim transposed,
  overlapped with the w1/w2 reduces on DVE. Linear Gauss-Seidel
  Sinkhorn, ITERS=4 with the last vs-update skipped (flow mass then =
  sum(w1), so logits normalize by 1/s1). w1/w2 are scaled by 256 so the
  fp16 iterates stay out of the subnormal range (cancels in 1/s1).
  KS=K*sim on gpsimd off the critical path.
Measured: ~152-155us (baseline v2: ~193-196us), rel err 4.96e-3.
"""

import os
import sys

import numpy as np

sys.path.insert(0, "/opt/trn_rl_repo")

import concourse.bass as bass
import concourse.bacc as bacc
import concourse.mybir as mybir
from concourse import tile
from concourse import masks
from concourse.bass_utils import run_bass_kernel_spmd

B_FULL, C, HW = 1024, 512, 49
NCORE = 8
BS = B_FULL // NCORE  # 128 batches per core
NCH = C // 128  # 4 chunks of 128 channels
AC = 2 * HW + 1  # 99 used slab cols per (batch, chunk): [Q | 1 | P]
AC2 = 100  # slab block pitch (pad col keeps every block 4B-aligned)
GRP = 32  # batches per load group
NGRP = BS // GRP
FB = 16  # batches per gs block / staged block
PAD = 50  # padded inner dim (4B-aligned fp16 rows)
FW = HW * PAD  # 2450
ITERS = 4
HALF = True  # skip last vs-update; normalize by s1
EXPB = 1.0 / 0.05  # K = exp(sim/eps) (the e^{-1/eps} factor cancels)
SIMPAD = -600.0
TEMP = 12.5 / HW
CPG = 2  # grams per PSUM bank / per copy (2 x 148 f32 fits a 2KB bank)
NBLK = BS // FB  # 8 staged blocks
H2B = 8  # batches per hop2 op
GW = 50  # gram output rows kept (Q rows 0:49 + ones row 49)
PITCH = 148  # per-batch gram row pitch: [99 gram cols | 49 dp cols]
WSCL = 256.0  # scale w1/w2 so the fp16 Sinkhorn iterates stay normal
#   (u ~ 2e-6 unscaled is deep fp16-subnormal; scaling both marginals by
#   the same factor leaves v unchanged, scales u and the flow mass by
#   WSCL, and cancels exactly in the final 1/s1 normalization)

f32 = mybir.dt.float32
f16 = mybir.dt.float16
Alu = mybir.AluOpType
Act = mybir.ActivationFunctionType
AxX = mybir.AxisListType.X

SLAB = GRP * NCH * AC2  # 12800 cols per group slab
SQW = GRP * NCH * PAD  # 6400 cols per squared-P slab (50-pitch, aligned)


def build_nc(debug=False):
    nc = bacc.Bacc(None, target_bir_lowering=False, debug=debug)
    aug = nc.declare_dram_parameter("aug", [NGRP, 128, SLAB], f16, isOutput=False)
    outp = nc.declare_dram_parameter("out", [BS, 1], f32, isOutput=True)
    dbg_names = os.environ.get("KDBG", "")
    dbgt = {}
    for nm in [x for x in dbg_names.split(",") if x]:
        dbgt[nm] = nc.declare_dram_parameter(
            f"dbg_{nm}", [BS, GW * PITCH if nm == "flatG" else FW], f16, isOutput=True
        )

    with tile.TileContext(nc) as tc:
        with (
            tc.tile_pool(name="big", bufs=1) as big,
            tc.tile_pool(name="stage", bufs=3) as stg,
            tc.tile_pool(name="sqst", bufs=4) as sqs,
            tc.tile_pool(name="gblk", bufs=4) as gcp,
            tc.tile_pool(name="small", bufs=1) as sml,
            tc.tile_pool(name="psum", bufs=7, space="PSUM") as pp,
            tc.tile_pool(name="dstg", bufs=1, space="DRAM") as dsp,
        ):
            flatG = big.tile([BS, GW * PITCH], f16, tag="flatG", name="flatG")

            simb = big.tile([BS, FW], f16, tag="sim", name="sim")
            Kb = big.tile([BS, FW], f16, tag="K", name="K")
            Ktb = big.tile([BS, FW], f16, tag="Kt", name="Kt")
            tb = big.tile([BS, FW], f16, tag="tb", name="tb")
            KS = big.tile([BS, FW], f16, tag="KS", name="KS")
            # pad prep: sim pads -> exp 0; Kt pad col stays 0
            nc.vector.memset(simb[:], SIMPAD)
            nc.vector.memset(Ktb[:], 0.0)

            def s_t(tag, w=HW, dt=f32):
                return sml.tile([BS, w], dt, tag=tag, name=tag)

            us = s_t("us", PAD, f16)
            vs = s_t("vs", PAD, f16)
            nc.vector.memset(us[:], 0.0)
            nc.vector.memset(vs[:], 0.0)

            # staged DRAM blocks, one per 16-batch gs block (separate tiles
            # so hop1(k+1) never serializes behind hop2(k))
            staged = [
                dsp.tile([GW, FB * PITCH], f16, tag=f"stgd{k}", name=f"stgd{k}")
                for k in range(NBLK)
            ]

            # ---------------- Phase A: load + gram + bounce-flatten ----------
            ths = []
            sqbs = []
            for g in range(NGRP):
                th = stg.tile([128, SLAB], f16, tag="h", name="hg")
                ths.append(th)
                # split every load into quarters: grams gate on ~2.3us
                # load granules, keeping PE idle gaps under the ~3.4us HAM
                # re-throttle window (cold PE runs matmuls at half clock)
                nspl = 4
                sw = SLAB // nspl
                for ss in range(nspl):
                    nc.sync.dma_start(
                        th[:, ss * sw : (ss + 1) * sw],
                        aug[g, :, ss * sw : (ss + 1) * sw],
                    )

            def h1eng(blk):
                # hop1 on gpsimd: each hop2 that waits on a hop1 is then
                # either behind it in the same FIFO or on another engine with
                # a monotone wait -- no cross-engine DMA wait cycles. Only
                # the very last hop1 moves to scalar: gpsimd's ring backlog
                # delays its completion ~7us right at the flatten tail.
                return nc.scalar if blk == NBLK - 1 else nc.gpsimd

            def h2eng(k):  # k = 2*blk + half
                if k == 15:
                    return nc.scalar  # FIFO right behind hop1(blk7)
                return (nc.gpsimd, nc.scalar, nc.sync)[k % 3]

            gs = None
            ps = None

            def emit_squares(g):
                # square the P columns of group g's slab on DVE (2x mode;
                # feeds the dp matmuls). Emitted one group ahead so the DVE
                # FIFO never makes the PE wait.
                th = ths[g]
                sqb = sqs.tile([128, SQW], f16, tag="sq", name="sqb")
                sqbs.append(sqb)
                thv = th[:].rearrange("c (n a) -> c n a", a=AC2)[:, :, HW + 1 : AC]
                sqv = sqb[:].rearrange("c (n a) -> c n a", a=PAD)[:, :, 0:HW]
                NB = GRP * NCH
                nsq = 4
                for hh in range(nsq):
                    nc.vector.tensor_mul(
                        sqv[:, hh * NB // nsq : (hh + 1) * NB // nsq, :],
                        thv[:, hh * NB // nsq : (hh + 1) * NB // nsq, :],
                        thv[:, hh * NB // nsq : (hh + 1) * NB // nsq, :],
                    )

            emit_squares(0)
            for g in range(NGRP):
                th = ths[g]
                for b in range(GRP):
                    if b == GRP // 2 and g + 1 < NGRP:
                        emit_squares(g + 1)
                    gb = g * GRP + b
                    blk, bb = gb // FB, gb % FB
                    if bb == 0:
                        gs = gcp.tile([GW, FB * PITCH], f16, tag="gs", name="gs")
                    cb = bb % CPG
                    if cb == 0:
                        ps = pp.tile([128, CPG * PITCH], f32, tag="gram", name="gram")
                    # start/stop must cover one partition range: widen all 4
                    # chunks or none (no widening only for the slab tail)
                    wid = 128 if b * (NCH * AC2) + (NCH - 1) * AC2 + 128 <= SLAB else AC
                    # one accumulation group for both column ranges: start
                    # clears has_written range-wide, so only the first MM may
                    # carry it and only the last carries stop
                    for j in range(NCH):
                        base = b * (NCH * AC2) + j * AC2
                        sqbase = b * (NCH * PAD) + j * PAD
                        nc.tensor.matmul(
                            ps[0:wid, cb * PITCH : cb * PITCH + AC],
                            th[:, base : base + wid],
                            th[:, base : base + AC],
                            start=(j == 0),
                            stop=False,
                        )
                        nc.tensor.matmul(
                            ps[0:wid, cb * PITCH + AC : (cb + 1) * PITCH],
                            th[:, base : base + wid],
                            sqbs[g][:, sqbase : sqbase + HW],
                            start=False,
                            stop=(j == NCH - 1),
                        )
                    if cb == CPG - 1:
                        nc.vector.tensor_copy(
                            gs[:, (bb - 1) * PITCH : (bb + 1) * PITCH], ps[0:GW, :]
                        )
                    if bb == FB - 1:
                        # hop1: gs block -> DRAM verbatim (50 fat descriptors)
                        h1eng(blk).dma_start(staged[blk][:, :], gs[:, :])
                        # hop2: DRAM -> batch-major flatG rows (2 ops of 8
                        # batches; 296B descriptors)
                        for half in range(FB // H2B):
                            b0 = half * H2B
                            sv = staged[blk][
                                :, b0 * PITCH : (b0 + H2B) * PITCH
                            ].copy()
                            sv.ap = mybir.VecI64Pair(
                                [[PITCH, H2B], [FB * PITCH, GW], [1, PITCH]]
                            )
                            gb0 = blk * FB + b0
                            h2eng(2 * blk + half).dma_start(
                                flatG[gb0 : gb0 + H2B, :], sv
                            )

            # ---------------- Phase B: fixups ----------------
            # flatG row layout per batch: rows i=0:49 are [QtQ(49) | c | QtP(49)
            # | x(49)] at pitch 148; row 49 is [sq(49) | C | sp(49) | dp(49)]
            R49 = (GW - 1) * PITCH

            def dview(col0, stride):
                v = flatG[:, col0 : col0 + 1].copy()
                v.ap = mybir.VecI64Pair([list(v.ap[0])] + [[stride, HW]])
                return v

            dq = dview(0, PITCH + 1)
            sq = flatG[:, R49 : R49 + HW]
            sp = flatG[:, R49 + HW + 1 : R49 + 2 * HW + 1]
            dp = flatG[:, R49 + AC : R49 + AC + HW]

            # batched rsqrt chain over [sq|sp], [dq|dp] (one pass, one
            # ACT round trip instead of two)
            d98 = s_t("d98", 2 * HW)
            s98 = s_t("s98", 2 * HW)
            t98, v98, iv98, n98 = (
                s_t("t98", 2 * HW),
                s_t("v98", 2 * HW),
                s_t("iv98", 2 * HW),
                s_t("n98", 2 * HW),
            )
            inv98 = s_t("inv98", 2 * HW, f16)
            with tc.high_priority():
                nc.vector.tensor_copy(d98[:, 0:HW], dq)
                nc.vector.tensor_copy(d98[:, HW : 2 * HW], dp)
                nc.vector.tensor_copy(s98[:, 0:HW], sq)
                nc.vector.tensor_copy(s98[:, HW : 2 * HW], sp)
                nc.vector.tensor_mul(t98[:], s98[:], s98[:])
                nc.vector.scalar_tensor_tensor(
                    v98[:], t98[:], -1.0 / C, d98[:], Alu.mult, Alu.add
                )
                nc.scalar.activation(t98[:], v98[:], Act.Sqrt)
                nc.vector.reciprocal(iv98[:], t98[:])
                nc.vector.tensor_mul(n98[:], iv98[:], iv98[:])
                nc.vector.tensor_mul(n98[:], n98[:], v98[:])
                nc.vector.tensor_scalar(n98[:], n98[:], -0.5, 1.5, Alu.mult, Alu.add)
                nc.vector.tensor_mul(inv98[:], iv98[:], n98[:])
            inq = inv98[:, 0:HW]
            inp_ = inv98[:, HW : 2 * HW]
            # preload the Exp table now (both Sqrt uses are done) so the
            # load hides under the sim-build DVE ops
            wrm = s_t("wrm", 1)
            nc.vector.memset(wrm[:], 1.0)
            nc.scalar.activation(wrm[:], wrm[:], Act.Exp)

            # sim = (qtp - sq x sp / C) * (inq x inp), in [49,50]-padded fp16
            G3 = flatG[:].rearrange("b (i k) -> b i k", k=PITCH)
            qtp3 = G3[:, 0:HW, PAD : PAD + HW]
            sim3 = simb[:].rearrange("b (q p) -> b q p", p=PAD)
            KS3 = KS[:].rearrange("b (q p) -> b q p", p=PAD)
            tb3 = tb[:].rearrange("b (q p) -> b q p", p=PAD)
            bq = inq.unsqueeze(2).broadcast_to([BS, HW, HW])
            bp = inp_.unsqueeze(1).broadcast_to([BS, HW, HW])
            bsq = sq.unsqueeze(2).broadcast_to([BS, HW, HW])
            bsp = sp.unsqueeze(1).broadcast_to([BS, HW, HW])
            # ssp2 = (-sq/C) x sp on gpsimd (flatG-only deps, starts the
            # moment flatG lands), concurrent with the DVE fixup chain
            s49 = s_t("s49", HW, f16)
            nc.vector.tensor_scalar_mul(s49[:], sq, -1.0 / C)
            bs49 = s49[:].unsqueeze(2).broadcast_to([BS, HW, HW])
            nc.gpsimd.tensor_mul(KS3[:, :, 0:HW], bs49, bsp)  # KS as scratch
            # nrm = inq x inp on DVE (needs the fixups); then centering and
            # the final product, both 2x. High priority: this is the
            # critical path into K-exp and the Sinkhorn.
            with tc.high_priority(offset=1000):
                nc.vector.tensor_mul(tb3[:, :, 0:HW], bq, bp)
                nc.vector.tensor_add(sim3[:, :, 0:HW], qtp3, KS3[:, :, 0:HW])
                nc.vector.tensor_mul(
                    sim3[:, :, 0:HW], sim3[:, :, 0:HW], tb3[:, :, 0:HW]
                )

            # K = exp(sim/eps); Kt via transposed read (ACT)
            nc.scalar.activation(Kb[:], simb[:], Act.Exp, scale=EXPB)
            simT = simb[:].rearrange("b (q p) -> b p q", p=PAD)[:, 0:HW, :]
            KtV = Ktb[:].rearrange("b (p q) -> b p q", q=PAD)[:, :, 0:HW]
            nc.scalar.activation(KtV, simT, Act.Exp, scale=EXPB)

            # w1/w2 reduces on DVE, overlapped with the ACT exps (w2r is
            # the slow strided one -- emitted last so it hides under K-exp)
            w1r, w2r = s_t("w1r"), s_t("w2r")
            w1f = s_t("w1f", HW, f16)
            w2f = s_t("w2f", HW, f16)
            s1s = s_t("s1s", 1)
            nc.vector.tensor_reduce(w1r[:], qtp3, axis=AxX, op=Alu.add)
            G3T = flatG[:].rearrange("b (i k) -> b k i", k=PITCH)
            qtpT = G3T[:, PAD : PAD + HW, 0:HW]  # [b, p, q(stride PITCH)]
            nc.vector.tensor_reduce(
                w2r[:, 0:25], qtpT[:, 0:25, :], axis=AxX, op=Alu.add
            )
            nc.vector.tensor_reduce(
                w2r[:, 25:HW], qtpT[:, 25:HW, :], axis=AxX, op=Alu.add
            )
            for wr, wf in ((w1r, w1f), (w2r, w2f)):
                nc.vector.tensor_scalar(wr[:], wr[:], WSCL / HW, 0.0, Alu.mult, Alu.max)
                nc.vector.tensor_scalar(wr[:], wr[:], 0.001 * WSCL, None, Alu.add)
                nc.vector.tensor_copy(wf[:], wr[:])
            nc.vector.tensor_reduce(s1s[:], w1r[:], axis=AxX, op=Alu.add)

            # KS = K * sim for the final logits (gpsimd, off critical path)
            nc.gpsimd.tensor_mul(KS[:], Kb[:], simb[:])

            # ---------------- Phase B: Sinkhorn ----------------
            K3 = Kb[:].rearrange("b (q p) -> b q p", p=PAD)
            Kt3 = Ktb[:].rearrange("b (p q) -> b p q", q=PAD)
            kv, rkv = s_t("kv"), s_t("rkv")
            bvs = vs[:].unsqueeze(1).broadcast_to([BS, HW, PAD])
            bus = us[:].unsqueeze(1).broadcast_to([BS, HW, PAD])
            for it in range(ITERS):
                # u-step: kv[q] = sum_p K[q,p] v[p]; u = w1 / kv
                if it == 0:
                    nc.vector.tensor_reduce(kv[:], K3, axis=AxX, op=Alu.add)
                else:
                    nc.vector.tensor_mul(tb3, K3, bvs)
                    nc.vector.tensor_reduce(kv[:], tb3, axis=AxX, op=Alu.add)
                nc.vector.reciprocal_approx_fast(rkv[:], kv[:])
                nc.vector.tensor_mul(us[:, 0:HW], w1f[:], rkv[:])
                if HALF and it == ITERS - 1:
                    break
                # v-step: ku[p] = sum_q K[q,p] u[q]; v = w2 / ku
                nc.vector.tensor_mul(tb3, Kt3, bus)
                nc.vector.tensor_reduce(kv[:], tb3, axis=AxX, op=Alu.add)
                nc.vector.reciprocal_approx_fast(rkv[:], kv[:])
                nc.vector.tensor_mul(vs[:, 0:HW], w2f[:], rkv[:])

            # ---------------- Phase B: logits ----------------
            lgr = s_t("lgr", 1)
            lgf = s_t("lgf", 1)
            nc.vector.tensor_mul(tb3, KS3, bvs)  # KS * vs
            nc.vector.tensor_reduce(kv[:], tb3, axis=AxX, op=Alu.add)
            nc.vector.tensor_mul(kv[:], kv[:], w1r[:])
            nc.vector.tensor_mul(kv[:], kv[:], rkv[:])
            nc.vector.tensor_reduce(lgr[:], kv[:], axis=AxX, op=Alu.add)
            nc.vector.reciprocal(rkv[:, 0:1], s1s[:])
            nc.vector.scalar_tensor_tensor(
                lgf[:], lgr[:], TEMP, rkv[:, 0:1], Alu.mult, Alu.mult
            )
            nc.sync.dma_start(outp[:, :], lgf[:])
            if "flatG" in dbgt:
                nc.sync.dma_start(dbgt["flatG"][:, :], flatG[:])
            if "sim" in dbgt:
                nc.sync.dma_start(dbgt["sim"][:, :], simb[:])
            if "K" in dbgt:
                nc.sync.dma_start(dbgt["K"][:, :], Kb[:])
            if "Kt" in dbgt:
                nc.sync.dma_start(dbgt["Kt"][:, :], Ktb[:])

    nc.compile()
    return nc


_NC = None


def _get_nc():
    global _NC
    if _NC is None:
        _NC = build_nc()
    return _NC


def _prep_in_maps(feature_map1, feature_map2):
    q = np.ascontiguousarray(np.asarray(feature_map1, dtype=np.float32)).reshape(
        B_FULL, C, HW
    )
    p = np.ascontiguousarray(np.asarray(feature_map2, dtype=np.float32)).reshape(
        B_FULL, C, HW
    )
    in_maps = []
    for i in range(NCORE):
        sl = slice(i * BS, (i + 1) * BS)
        a32 = np.empty((NGRP, 128, GRP, NCH, AC2), np.float32)
        a32[..., HW] = 1.0
        a32[..., AC2 - 1] = 0.0
        qc = q[sl].reshape(NGRP, GRP, NCH, 128, HW).transpose(0, 3, 1, 2, 4)
        pc = p[sl].reshape(NGRP, GRP, NCH, 128, HW).transpose(0, 3, 1, 2, 4)
        a32[..., 0:HW] = qc
        a32[..., HW + 1 : AC] = pc
        in_maps.append({"aug": a32.astype(np.float16).reshape(NGRP, 128, SLAB)})
    return in_maps


def run(feature_map1, feature_map2, trace=False):
    in_maps = _prep_in_maps(feature_map1, feature_map2)
    nc = _get_nc()
    res = run_bass_kernel_spmd(nc, in_maps, core_ids=list(range(NCORE)), trace=trace)
    out = np.concatenate(
        [np.asarray(res.results[i]["out"]).reshape(BS) for i in range(NCORE)]
    ).astype(np.float32)
    return out, res


def kernel(feature_map1, feature_map2):
    out, _ = run(feature_map1, feature_map2, trace=False)
    return out


# revision 36
# speedup vs baseline: 1.1711x; 1.0501x over previous
"""DeepEMD Trainium2 kernel: batched 49x49 entropic-OT (Sinkhorn) similarity.

Final strategy (8 NeuronCores, data-parallel over batch; 128 batches/core):
- Host prepacks per (group of 32 batches) slabs A = [Q | 1 | P | pad]
  (128ch x 100, fp16). The ones column sits at col 49 so every gram row
  phase B needs lands in contiguous output rows 0:50; the pad col keeps
  every block 4B-aligned so DVE ops hit the 2x packed mode.
- Phase A: 4 group loads in 2.3us quarter-granules (keeps PE idle gaps
  under the ~3.4us HAM re-throttle window). Per batch, one PSUM
  accumulation group of 8 matmuls: 4 chunks x (W^T M1 + W^T M2) with
  W = [Q|1|P] widened to 128 cols (FWL), M1 = [Q|1|P], M2 = P^2.
  P^2 is squared on DVE (2x mode), one group ahead of use. Output rows
  0:49 = [QtQ (diag dq) | . | QtP]; row 49 = [sq | C | sp | dp].
- Gram flatten (gram-major -> batch-major flatG[128, 50*148]) via a DRAM
  bounce: hop1 copies each 16-batch gs block to DRAM verbatim (50 fat
  4736B descriptors, all on the gpsimd queue so hop2 waits are FIFO-local
  or monotone), hop2 reads it back batch-major in 8-batch ops (296B
  descriptors, round-robin over all 3 DMA queues). DMA queues generate
  descriptors at ~10-15ns each, so descriptor count is the binding
  resource: the row-0:50 trim halves it vs a full-gram flatten (6400 vs
  12672) and the bounce replaces 128 per-batch SBUF->SBUF ops (~1.7us
  per-op overhead each) with 24 ops.
- Phase B: all big tensors fp16 in a [49, 50]-padded layout (2x DVE).
  Batched rsqrt fixups (one ACT round trip); sim = (qtp - sq x sp/C) *
  (inq x inp); K = exp(20*sim) directly (the e^{-1/eps} rescale cancels
  in the final normalization); Kt via ACT reading sim transposed,
  overlapped with the w1/w2 reduces on DVE. Linear Gauss-Seidel
  Sinkhorn, ITERS=4 with the last vs-update skipped (flow mass then =
  sum(w1), so logits normalize by 1/s1). w1/w2 are scaled by 256 so the
  fp16 iterates stay out of the subnormal range (cancels in 1/s1).
  KS=K*sim on gpsimd off the critical path.
Measured: ~152-155us (baseline v2: ~193-196us), rel err 4.96e-3.
"""

import os
import sys

import numpy as np

sys.path.insert(0, "/opt/trn_rl_repo")

import concourse.bass as bass
import concourse.bacc as bacc
import concourse.mybir as mybir
from concourse import tile
from concourse import masks
from concourse.bass_utils import run_bass_kernel_spmd

B_FULL, C, HW = 1024, 512, 49
NCORE = 8
BS = B_FULL // NCORE  # 128 batches per core
NCH = C // 128  # 4 chunks of 128 channels
AC = 2 * HW + 1  # 99 used slab cols per (batch, chunk): [Q | 1 | P]
AC2 = 100  # slab block pitch (pad col keeps every block 4B-aligned)
GRP = 32  # batches per load group
NGRP = BS // GRP
FB = 16  # batches per gs block / staged block
PAD = 50  # padded inner dim (4B-aligned fp16 rows)
FW = HW * PAD  # 2450
ITERS = 4
HALF = True  # skip last vs-update; normalize by s1
EXPB = 1.0 / 0.05  # K = exp(sim/eps) (the e^{-1/eps} factor cancels)
SIMPAD = -600.0
TEMP = 12.5 / HW
CPG = 2  # grams per PSUM bank / per copy (2 x 148 f32 fits a 2KB bank)
NBLK = BS // FB  # 8 staged blocks
H2B = 8  # batches per hop2 op
GW = 50  # gram output rows kept (Q rows 0:49 + ones row 49)
PITCH = 148  # per-batch gram row pitch: [99 gram cols | 49 dp cols]
WSCL = 256.0  # scale w1/w2 so the fp16 Sinkhorn iterates stay normal
#   (u ~ 2e-6 unscaled is deep fp16-subnormal; scaling both marginals by
#   the same factor leaves v unchanged, scales u and the flow mass by
#   WSCL, and cancels exactly in the final 1/s1 normalization)

f32 = mybir.dt.float32
f16 = mybir.dt.float16
Alu = mybir.AluOpType
Act = mybir.ActivationFunctionType
AxX = mybir.AxisListType.X

SLAB = GRP * NCH * AC2  # 12800 cols per group slab
SQW = GRP * NCH * PAD  # 6400 cols per squared-P slab (50-pitch, aligned)


def build_nc(debug=False):
    nc = bacc.Bacc(None, target_bir_lowering=False, debug=debug)
    aug = nc.declare_dram_parameter("aug", [NGRP, 128, SLAB], f16, isOutput=False)
    outp = nc.declare_dram_parameter("out", [BS, 1], f32, isOutput=True)
    dbg_names = os.environ.get("KDBG", "")
    dbgt = {}
    for nm in [x for x in dbg_names.split(",") if x]:
        dbgt[nm] = nc.declare_dram_parameter(
            f"dbg_{nm}", [BS, GW * PITCH if nm == "flatG" else FW], f16, isOutput=True
        )

    with tile.TileContext(nc) as tc:
        with (
            tc.tile_pool(name="big", bufs=1) as big,
            tc.tile_pool(name="stage", bufs=3) as stg,
            tc.tile_pool(name="sqst", bufs=4) as sqs,
            tc.tile_pool(name="gblk", bufs=4) as gcp,
            tc.tile_pool(name="small", bufs=1) as sml,
            tc.tile_pool(name="psum", bufs=7, space="PSUM") as pp,
            tc.tile_pool(name="dstg", bufs=1, space="DRAM") as dsp,
        ):
            flatG = big.tile([BS, GW * PITCH], f16, tag="flatG", name="flatG")

            simb = big.tile([BS, FW], f16, tag="sim", name="sim")
            Kb = big.tile([BS, FW], f16, tag="K", name="K")
            Ktb = big.tile([BS, FW], f16, tag="Kt", name="Kt")
            tb = big.tile([BS, FW], f16, tag="tb", name="tb")
            KS = big.tile([BS, FW], f16, tag="KS", name="KS")
            # pad prep: sim pads -> exp 0; Kt pad col stays 0
            nc.vector.memset(simb[:], SIMPAD)
            nc.vector.memset(Ktb[:], 0.0)

            def s_t(tag, w=HW, dt=f32):
                return sml.tile([BS, w], dt, tag=tag, name=tag)

            us = s_t("us", PAD, f16)
            vs = s_t("vs", PAD, f16)
            nc.vector.memset(us[:], 0.0)
            nc.vector.memset(vs[:], 0.0)
            # warm the ACT Sqrt table at t=0 (scalar is idle all of phase A;
            # a lazy load would sit on the phase-B critical path)
            wrm0 = s_t("wrm0", 1)
            nc.vector.memset(wrm0[:], 1.0)
            nc.scalar.activation(wrm0[:], wrm0[:], Act.Sqrt)

            # staged DRAM blocks, one per 16-batch gs block (separate tiles
            # so hop1(k+1) never serializes behind hop2(k))
            staged = [
                dsp.tile([GW, FB * PITCH], f16, tag=f"stgd{k}", name=f"stgd{k}")
                for k in range(NBLK)
            ]

            # ---------------- Phase A: load + gram + bounce-flatten ----------
            ths = []
            sqbs = []
            for g in range(NGRP):
                th = stg.tile([128, SLAB], f16, tag="h", name="hg")
                ths.append(th)
                # split every load into quarters: grams gate on ~2.3us
                # load granules, keeping PE idle gaps under the ~3.4us HAM
                # re-throttle window (cold PE runs matmuls at half clock)
                nspl = 4
                sw = SLAB // nspl
                for ss in range(nspl):
                    nc.sync.dma_start(
                        th[:, ss * sw : (ss + 1) * sw],
                        aug[g, :, ss * sw : (ss + 1) * sw],
                    )

            def h1eng(blk):
                # all hop1 on gpsimd: each hop2 that waits on a hop1 is then
                # either behind it in the same FIFO or on another engine with
                # a monotone wait -- no cross-engine DMA wait cycles
                return nc.gpsimd

            def h2eng(k):  # k = 2*blk + half
                return (nc.gpsimd, nc.scalar, nc.sync)[k % 3]

            gs = None
            ps = None

            def emit_squares(g):
                # square the P columns of group g's slab on DVE (2x mode;
                # feeds the dp matmuls). Emitted one group ahead so the DVE
                # FIFO never makes the PE wait.
                th = ths[g]
                sqb = sqs.tile([128, SQW], f16, tag="sq", name="sqb")
                sqbs.append(sqb)
                thv = th[:].rearrange("c (n a) -> c n a", a=AC2)[:, :, HW + 1 : AC]
                sqv = sqb[:].rearrange("c (n a) -> c n a", a=PAD)[:, :, 0:HW]
                NB = GRP * NCH
                nsq = 4
                for hh in range(nsq):
                    nc.vector.tensor_mul(
                        sqv[:, hh * NB // nsq : (hh + 1) * NB // nsq, :],
                        thv[:, hh * NB // nsq : (hh + 1) * NB // nsq, :],
                        thv[:, hh * NB // nsq : (hh + 1) * NB // nsq, :],
                    )

            emit_squares(0)
            for g in range(NGRP):
                th = ths[g]
                for b in range(GRP):
                    if b == GRP // 2 and g + 1 < NGRP:
                        emit_squares(g + 1)
                    gb = g * GRP + b
                    blk, bb = gb // FB, gb % FB
                    if bb == 0:
                        gs = gcp.tile([GW, FB * PITCH], f16, tag="gs", name="gs")
                    cb = bb % CPG
                    if cb == 0:
                        ps = pp.tile([128, CPG * PITCH], f32, tag="gram", name="gram")
                    # start/stop must cover one partition range: widen all 4
                    # chunks or none (no widening only for the slab tail)
                    wid = 128 if b * (NCH * AC2) + (NCH - 1) * AC2 + 128 <= SLAB else AC
                    # one accumulation group for both column ranges: start
                    # clears has_written range-wide, so only the first MM may
                    # carry it and only the last carries stop
                    for j in range(NCH):
                        base = b * (NCH * AC2) + j * AC2
                        sqbase = b * (NCH * PAD) + j * PAD
                        nc.tensor.matmul(
                            ps[0:wid, cb * PITCH : cb * PITCH + AC],
                            th[:, base : base + wid],
                            th[:, base : base + AC],
                            start=(j == 0),
                            stop=False,
                        )
                        nc.tensor.matmul(
                            ps[0:wid, cb * PITCH + AC : (cb + 1) * PITCH],
                            th[:, base : base + wid],
                            sqbs[g][:, sqbase : sqbase + HW],
                            start=False,
                            stop=(j == NCH - 1),
                        )
                    if cb == CPG - 1:
                        nc.vector.tensor_copy(
                            gs[:, (bb - 1) * PITCH : (bb + 1) * PITCH], ps[0:GW, :]
                        )
                    if bb == FB - 1:
                        # hop1: gs block -> DRAM verbatim (50 fat descriptors)
                        h1eng(blk).dma_start(staged[blk][:, :], gs[:, :])
                        # hop2: DRAM -> batch-major flatG rows (2 ops of 8
                        # batches; 296B descriptors)
                        for half in range(FB // H2B):
                            b0 = half * H2B
                            sv = staged[blk][
                                :, b0 * PITCH : (b0 + H2B) * PITCH
                            ].copy()
                            sv.ap = mybir.VecI64Pair(
                                [[PITCH, H2B], [FB * PITCH, GW], [1, PITCH]]
                            )
                            gb0 = blk * FB + b0
                            h2eng(2 * blk + half).dma_start(
                                flatG[gb0 : gb0 + H2B, :], sv
                            )

            # ---------------- Phase B: fixups ----------------
            # flatG row layout per batch: rows i=0:49 are [QtQ(49) | c | QtP(49)
            # | x(49)] at pitch 148; row 49 is [sq(49) | C | sp(49) | dp(49)]
            R49 = (GW - 1) * PITCH

            def dview(col0, stride):
                v = flatG[:, col0 : col0 + 1].copy()
                v.ap = mybir.VecI64Pair([list(v.ap[0])] + [[stride, HW]])
                return v

            dq = dview(0, PITCH + 1)
            sq = flatG[:, R49 : R49 + HW]
            sp = flatG[:, R49 + HW + 1 : R49 + 2 * HW + 1]
            dp = flatG[:, R49 + AC : R49 + AC + HW]

            # batched rsqrt chain over [sq|sp], [dq|dp] (one pass, one
            # ACT round trip instead of two)
            d98 = s_t("d98", 2 * HW)
            s98 = s_t("s98", 2 * HW)
            t98, v98, iv98, n98 = (
                s_t("t98", 2 * HW),
                s_t("v98", 2 * HW),
                s_t("iv98", 2 * HW),
                s_t("n98", 2 * HW),
            )
            inv98 = s_t("inv98", 2 * HW, f16)
            with tc.high_priority():
                nc.vector.tensor_copy(d98[:, 0:HW], dq)
                nc.vector.tensor_copy(d98[:, HW : 2 * HW], dp)
                nc.vector.tensor_copy(s98[:, 0:HW], sq)
                nc.vector.tensor_copy(s98[:, HW : 2 * HW], sp)
                nc.vector.tensor_mul(t98[:], s98[:], s98[:])
                nc.vector.scalar_tensor_tensor(
                    v98[:], t98[:], -1.0 / C, d98[:], Alu.mult, Alu.add
                )
                nc.scalar.activation(t98[:], v98[:], Act.Sqrt)
                nc.vector.reciprocal(iv98[:], t98[:])
                nc.vector.tensor_mul(n98[:], iv98[:], iv98[:])
                nc.vector.tensor_mul(n98[:], n98[:], v98[:])
                nc.vector.tensor_scalar(n98[:], n98[:], -0.5, 1.5, Alu.mult, Alu.add)
                nc.vector.tensor_mul(inv98[:], iv98[:], n98[:])
            inq = inv98[:, 0:HW]
            inp_ = inv98[:, HW : 2 * HW]
            # preload the Exp table now (both Sqrt uses are done) so the
            # load hides under the sim-build DVE ops
            wrm = s_t("wrm", 1)
            nc.vector.memset(wrm[:], 1.0)
            nc.scalar.activation(wrm[:], wrm[:], Act.Exp)

            # sim = (qtp - sq x sp / C) * (inq x inp), in [49,50]-padded fp16
            G3 = flatG[:].rearrange("b (i k) -> b i k", k=PITCH)
            qtp3 = G3[:, 0:HW, PAD : PAD + HW]
            sim3 = simb[:].rearrange("b (q p) -> b q p", p=PAD)
            KS3 = KS[:].rearrange("b (q p) -> b q p", p=PAD)
            tb3 = tb[:].rearrange("b (q p) -> b q p", p=PAD)
            bq = inq.unsqueeze(2).broadcast_to([BS, HW, HW])
            bp = inp_.unsqueeze(1).broadcast_to([BS, HW, HW])
            bsq = sq.unsqueeze(2).broadcast_to([BS, HW, HW])
            bsp = sp.unsqueeze(1).broadcast_to([BS, HW, HW])
            # ssp2 = (-sq/C) x sp on gpsimd (flatG-only deps, starts the
            # moment flatG lands), concurrent with the DVE fixup chain
            s49 = s_t("s49", HW, f16)
            nc.vector.tensor_scalar_mul(s49[:], sq, -1.0 / C)
            bs49 = s49[:].unsqueeze(2).broadcast_to([BS, HW, HW])
            nc.gpsimd.tensor_mul(KS3[:, :, 0:HW], bs49, bsp)  # KS as scratch
            # nrm = inq x inp on DVE (needs the fixups); then centering and
            # the final product, both 2x. High priority: this is the
            # critical path into K-exp and the Sinkhorn.
            with tc.high_priority(offset=1000):
                nc.vector.tensor_mul(tb3[:, :, 0:HW], bq, bp)
                nc.vector.tensor_add(sim3[:, :, 0:HW], qtp3, KS3[:, :, 0:HW])
                nc.vector.tensor_mul(
                    sim3[:, :, 0:HW], sim3[:, :, 0:HW], tb3[:, :, 0:HW]
                )

            # K = exp(sim/eps); Kt via transposed read (ACT)
            nc.scalar.activation(Kb[:], simb[:], Act.Exp, scale=EXPB)
            simT = simb[:].rearrange("b (q p) -> b p q", p=PAD)[:, 0:HW, :]
            KtV = Ktb[:].rearrange("b (p q) -> b p q", q=PAD)[:, :, 0:HW]
            nc.scalar.activation(KtV, simT, Act.Exp, scale=EXPB)

            # w1/w2 reduces on DVE, overlapped with the ACT exps (w2r is
            # the slow strided one -- emitted last so it hides under K-exp)
            w1r, w2r = s_t("w1r"), s_t("w2r")
            w1f = s_t("w1f", HW, f16)
            w2f = s_t("w2f", HW, f16)
            s1s = s_t("s1s", 1)
            nc.vector.tensor_reduce(w1r[:], qtp3, axis=AxX, op=Alu.add)
            G3T = flatG[:].rearrange("b (i k) -> b k i", k=PITCH)
            qtpT = G3T[:, PAD : PAD + HW, 0:HW]  # [b, p, q(stride PITCH)]
            nc.vector.tensor_reduce(
                w2r[:, 0:25], qtpT[:, 0:25, :], axis=AxX, op=Alu.add
            )
            nc.vector.tensor_reduce(
                w2r[:, 25:HW], qtpT[:, 25:HW, :], axis=AxX, op=Alu.add
            )
            for wr, wf in ((w1r, w1f), (w2r, w2f)):
                nc.vector.tensor_scalar(wr[:], wr[:], WSCL / HW, 0.0, Alu.mult, Alu.max)
                nc.vector.tensor_scalar(wr[:], wr[:], 0.001 * WSCL, None, Alu.add)
                nc.vector.tensor_copy(wf[:], wr[:])
            nc.vector.tensor_reduce(s1s[:], w1r[:], axis=AxX, op=Alu.add)

            # KS = K * sim for the final logits (gpsimd, off critical path)
            nc.gpsimd.tensor_mul(KS[:], Kb[:], simb[:])

            # ---------------- Phase B: Sinkhorn ----------------
            K3 = Kb[:].rearrange("b (q p) -> b q p", p=PAD)
            Kt3 = Ktb[:].rearrange("b (p q) -> b p q", q=PAD)
            kv, rkv = s_t("kv"), s_t("rkv")
            bvs = vs[:].unsqueeze(1).broadcast_to([BS, HW, PAD])
            bus = us[:].unsqueeze(1).broadcast_to([BS, HW, PAD])
            for it in range(ITERS):
                # u-step: kv[q] = sum_p K[q,p] v[p]; u = w1 / kv
                if it == 0:
                    nc.vector.tensor_reduce(kv[:], K3, axis=AxX, op=Alu.add)
                else:
                    nc.vector.tensor_mul(tb3, K3, bvs)
                    nc.vector.tensor_reduce(kv[:], tb3, axis=AxX, op=Alu.add)
                nc.vector.reciprocal_approx_fast(rkv[:], kv[:])
                nc.vector.tensor_mul(us[:, 0:HW], w1f[:], rkv[:])
                if HALF and it == ITERS - 1:
                    break
                # v-step: ku[p] = sum_q K[q,p] u[q]; v = w2 / ku
                nc.vector.tensor_mul(tb3, Kt3, bus)
                nc.vector.tensor_reduce(kv[:], tb3, axis=AxX, op=Alu.add)
                nc.vector.reciprocal_approx_fast(rkv[:], kv[:])
                nc.vector.tensor_mul(vs[:, 0:HW], w2f[:], rkv[:])

            # ---------------- Phase B: logits ----------------
            lgr = s_t("lgr", 1)
            lgf = s_t("lgf", 1)
            nc.vector.tensor_mul(tb3, KS3, bvs)  # KS * vs
            nc.vector.tensor_reduce(kv[:], tb3, axis=AxX, op=Alu.add)
            nc.vector.tensor_mul(kv[:], kv[:], w1r[:])
            nc.vector.tensor_mul(kv[:], kv[:], rkv[:])
            nc.vector.tensor_reduce(lgr[:], kv[:], axis=AxX, op=Alu.add)
            nc.vector.reciprocal(rkv[:, 0:1], s1s[:])
            nc.vector.scalar_tensor_tensor(
                lgf[:], lgr[:], TEMP, rkv[:, 0:1], Alu.mult, Alu.mult
            )
            nc.sync.dma_start(outp[:, :], lgf[:])
            if "flatG" in dbgt:
                nc.sync.dma_start(dbgt["flatG"][:, :], flatG[:])
            if "sim" in dbgt:
                nc.sync.dma_start(dbgt["sim"][:, :], simb[:])
            if "K" in dbgt:
                nc.sync.dma_start(dbgt["K"][:, :], Kb[:])
            if "Kt" in dbgt:
                nc.sync.dma_start(dbgt["Kt"][:, :], Ktb[:])

    nc.compile()
    return nc


_NC = None


def _get_nc():
    global _NC
    if _NC is None:
        _NC = build_nc()
    return _NC


def _prep_in_maps(feature_map1, feature_map2):
    q = np.ascontiguousarray(np.asarray(feature_map1, dtype=np.float32)).reshape(
        B_FULL, C, HW
    )
    p = np.ascontiguousarray(np.asarray(feature_map2, dtype=np.float32)).reshape(
        B_FULL, C, HW
    )
    in_maps = []
    for i in range(NCORE):
        sl = slice(i * BS, (i + 1) * BS)
        a32 = np.empty((NGRP, 128, GRP, NCH, AC2), np.float32)
        a32[..., HW] = 1.0
        a32[..., AC2 - 1] = 0.0
        qc = q[sl].reshape(NGRP, GRP, NCH, 128, HW).transpose(0, 3, 1, 2, 4)
        pc = p[sl].reshape(NGRP, GRP, NCH, 128, HW).transpose(0, 3, 1, 2, 4)
        a32[..., 0:HW] = qc
        a32[..., HW + 1 : AC] = pc
        in_maps.append({"aug": a32.astype(np.float16).reshape(NGRP, 128, SLAB)})
    return in_maps


def run(feature_map1, feature_map2, trace=False):
    in_maps = _prep_in_maps(feature_map1, feature_map2)
    nc = _get_nc()
    res = run_bass_kernel_spmd(nc, in_maps, core_ids=list(range(NCORE)), trace=trace)
    out = np.concatenate(
        [np.asarray(res.results[i]["out"]).reshape(BS) for i in range(NCORE)]
    ).astype(np.float32)
    return out, res


def kernel(feature_map1, feature_map2):
    out, _ = run(feature_map1, feature_map2, trace=False)
    return out


# revision 38
# speedup vs baseline: 1.2324x; 1.0524x over previous
"""DeepEMD Trainium2 kernel: batched 49x49 entropic-OT (Sinkhorn) similarity.

Final strategy (8 NeuronCores, data-parallel over batch; 128 batches/core):
- Host prepacks per (group of 32 batches) slabs A = [Q | 1 | P | pad]
  (128ch x 100, fp16). The ones column sits at col 49 so every gram row
  phase B needs lands in contiguous output rows 0:50; the pad col keeps
  every block 4B-aligned so DVE ops hit the 2x packed mode.
- Phase A: 4 group loads in 2.3us quarter-granules (keeps PE idle gaps
  under the ~3.4us HAM re-throttle window). Per batch, one PSUM
  accumulation group of 8 matmuls: 4 chunks x (W^T M1 + W^T M2) with
  W = [Q|1|P] widened to 128 cols (FWL), M1 = [Q|1|P], M2 = P^2.
  P^2 is squared on DVE (2x mode), one group ahead of use. Output rows
  0:49 = [QtQ (diag dq) | . | QtP]; row 49 = [sq | C | sp | dp].
- Gram flatten (gram-major -> batch-major flatG[128, 50*148]) via a DRAM
  bounce: hop1 copies each 16-batch gs block to DRAM verbatim (50 fat
  4736B descriptors, all on the gpsimd queue so hop2 waits are FIFO-local
  or monotone), hop2 reads it back batch-major in 8-batch ops (296B
  descriptors, round-robin over all 3 DMA queues). DMA queues generate
  descriptors at ~10-15ns each, so descriptor count is the binding
  resource: the row-0:50 trim halves it vs a full-gram flatten (6400 vs
  12672) and the bounce replaces 128 per-batch SBUF->SBUF ops (~1.7us
  per-op overhead each) with 24 ops.
- Phase B: all big tensors fp16 in a [49, 50]-padded layout (2x DVE).
  Batched rsqrt fixups (one ACT round trip); sim = (qtp - sq x sp/C) *
  (inq x inp); K = exp(20*sim) directly (the e^{-1/eps} rescale cancels
  in the final normalization); Kt via ACT reading sim transposed,
  overlapped with the w1/w2 reduces on DVE. Linear Gauss-Seidel
  Sinkhorn, ITERS=4 with the last vs-update skipped (flow mass then =
  sum(w1), so logits normalize by 1/s1). w1/w2 are scaled by 256 so the
  fp16 iterates stay out of the subnormal range (cancels in 1/s1).
  KS=K*sim on gpsimd off the critical path.
Measured: ~152-155us (baseline v2: ~193-196us), rel err 4.96e-3.
"""

import os
import sys

import numpy as np

sys.path.insert(0, "/opt/trn_rl_repo")

import concourse.bass as bass
import concourse.bacc as bacc
import concourse.mybir as mybir
from concourse import tile
from concourse import masks
from concourse.bass_utils import run_bass_kernel_spmd

B_FULL, C, HW = 1024, 512, 49
NCORE = 8
BS = B_FULL // NCORE  # 128 batches per core
NCH = C // 128  # 4 chunks of 128 channels
AC = 2 * HW + 1  # 99 used slab cols per (batch, chunk): [Q | 1 | P]
AC2 = 100  # slab block pitch (pad col keeps every block 4B-aligned)
GRP = 32  # batches per load group
NGRP = BS // GRP
FB = 16  # batches per gs block / staged block
PAD = 50  # padded inner dim (4B-aligned fp16 rows)
FW = HW * PAD  # 2450
ITERS = 3
HALF = True  # skip last vs-update; normalize by s1
EXPB = 1.0 / 0.05  # K = exp(sim/eps) (the e^{-1/eps} factor cancels)
SIMPAD = -600.0
TEMP = 12.5 / HW
CPG = 2  # grams per PSUM bank / per copy (2 x 148 f32 fits a 2KB bank)
NBLK = BS // FB  # 8 staged blocks
H2B = 8  # batches per hop2 op
GW = 50  # gram output rows kept (Q rows 0:49 + ones row 49)
PITCH = 148  # per-batch gram row pitch: [99 gram cols | 49 dp cols]
WSCL = 4096.0  # scale w1/w2 so the fp16 Sinkhorn iterates stay normal
#   (u ~ 2e-6 unscaled is deep fp16-subnormal; scaling both marginals by
#   the same factor leaves v unchanged, scales u and the flow mass by
#   WSCL, and cancels exactly in the final 1/s1 normalization)

f32 = mybir.dt.float32
f16 = mybir.dt.float16
Alu = mybir.AluOpType
Act = mybir.ActivationFunctionType
AxX = mybir.AxisListType.X

SLAB = GRP * NCH * AC2  # 12800 cols per group slab
SQW = GRP * NCH * PAD  # 6400 cols per squared-P slab (50-pitch, aligned)


def build_nc(debug=False):
    nc = bacc.Bacc(None, target_bir_lowering=False, debug=debug)
    aug = nc.declare_dram_parameter("aug", [NGRP, 128, SLAB], f16, isOutput=False)
    outp = nc.declare_dram_parameter("out", [BS, 1], f32, isOutput=True)
    dbg_names = os.environ.get("KDBG", "")
    dbgt = {}
    for nm in [x for x in dbg_names.split(",") if x]:
        dbgt[nm] = nc.declare_dram_parameter(
            f"dbg_{nm}", [BS, GW * PITCH if nm == "flatG" else FW], f16, isOutput=True
        )

    with tile.TileContext(nc) as tc:
        with (
            tc.tile_pool(name="big", bufs=1) as big,
            tc.tile_pool(name="stage", bufs=3) as stg,
            tc.tile_pool(name="sqst", bufs=4) as sqs,
            tc.tile_pool(name="gblk", bufs=4) as gcp,
            tc.tile_pool(name="small", bufs=1) as sml,
            tc.tile_pool(name="psum", bufs=7, space="PSUM") as pp,
            tc.tile_pool(name="dstg", bufs=1, space="DRAM") as dsp,
        ):
            flatG = big.tile([BS, GW * PITCH], f16, tag="flatG", name="flatG")

            simb = big.tile([BS, FW], f16, tag="sim", name="sim")
            Kb = big.tile([BS, FW], f16, tag="K", name="K")
            Ktb = big.tile([BS, FW], f16, tag="Kt", name="Kt")
            tb = big.tile([BS, FW], f16, tag="tb", name="tb")
            KS = big.tile([BS, FW], f16, tag="KS", name="KS")
            # pad prep: sim pads -> exp 0; Kt pad col stays 0
            nc.vector.memset(simb[:], SIMPAD)
            nc.vector.memset(Ktb[:], 0.0)

            def s_t(tag, w=HW, dt=f32):
                return sml.tile([BS, w], dt, tag=tag, name=tag)

            us = s_t("us", PAD, f16)
            vs = s_t("vs", PAD, f16)
            nc.vector.memset(us[:], 0.0)
            nc.vector.memset(vs[:], 0.0)

            # staged DRAM blocks, one per 16-batch gs block (separate tiles
            # so hop1(k+1) never serializes behind hop2(k))
            staged = [
                dsp.tile([GW, FB * PITCH], f16, tag=f"stgd{k}", name=f"stgd{k}")
                for k in range(NBLK)
            ]

            # ---------------- Phase A: load + gram + bounce-flatten ----------
            ths = []
            sqbs = []
            for g in range(NGRP):
                th = stg.tile([128, SLAB], f16, tag="h", name="hg")
                ths.append(th)
                # split every load into quarters: grams gate on ~2.3us
                # load granules, keeping PE idle gaps under the ~3.4us HAM
                # re-throttle window (cold PE runs matmuls at half clock)
                nspl = 4
                sw = SLAB // nspl
                for ss in range(nspl):
                    nc.sync.dma_start(
                        th[:, ss * sw : (ss + 1) * sw],
                        aug[g, :, ss * sw : (ss + 1) * sw],
                    )

            def h1eng(blk):
                # all hop1 on gpsimd: each hop2 that waits on a hop1 is then
                # either behind it in the same FIFO or on another engine with
                # a monotone wait -- no cross-engine DMA wait cycles
                return nc.gpsimd

            def h2eng(k):  # k = 2*blk + half
                return (nc.gpsimd, nc.scalar, nc.sync)[k % 3]

            gs = None
            ps = None

            def emit_squares(g):
                # square the P columns of group g's slab on DVE (2x mode;
                # feeds the dp matmuls). Emitted one group ahead so the DVE
                # FIFO never makes the PE wait.
                th = ths[g]
                sqb = sqs.tile([128, SQW], f16, tag="sq", name="sqb")
                sqbs.append(sqb)
                thv = th[:].rearrange("c (n a) -> c n a", a=AC2)[:, :, HW + 1 : AC]
                sqv = sqb[:].rearrange("c (n a) -> c n a", a=PAD)[:, :, 0:HW]
                NB = GRP * NCH
                nsq = 4
                for hh in range(nsq):
                    nc.vector.tensor_mul(
                        sqv[:, hh * NB // nsq : (hh + 1) * NB // nsq, :],
                        thv[:, hh * NB // nsq : (hh + 1) * NB // nsq, :],
                        thv[:, hh * NB // nsq : (hh + 1) * NB // nsq, :],
                    )

            emit_squares(0)
            for g in range(NGRP):
                th = ths[g]
                for b in range(GRP):
                    if b == GRP // 2 and g + 1 < NGRP:
                        emit_squares(g + 1)
                    gb = g * GRP + b
                    blk, bb = gb // FB, gb % FB
                    if bb == 0:
                        gs = gcp.tile([GW, FB * PITCH], f16, tag="gs", name="gs")
                    cb = bb % CPG
                    if cb == 0:
                        ps = pp.tile([128, CPG * PITCH], f32, tag="gram", name="gram")
                    # start/stop must cover one partition range: widen all 4
                    # chunks or none (no widening only for the slab tail)
                    wid = 128 if b * (NCH * AC2) + (NCH - 1) * AC2 + 128 <= SLAB else AC
                    # one accumulation group for both column ranges: start
                    # clears has_written range-wide, so only the first MM may
                    # carry it and only the last carries stop
                    for j in range(NCH):
                        base = b * (NCH * AC2) + j * AC2
                        sqbase = b * (NCH * PAD) + j * PAD
                        nc.tensor.matmul(
                            ps[0:wid, cb * PITCH : cb * PITCH + AC],
                            th[:, base : base + wid],
                            th[:, base : base + AC],
                            start=(j == 0),
                            stop=False,
                        )
                        nc.tensor.matmul(
                            ps[0:wid, cb * PITCH + AC : (cb + 1) * PITCH],
                            th[:, base : base + wid],
                            sqbs[g][:, sqbase : sqbase + HW],
                            start=False,
                            stop=(j == NCH - 1),
                        )
                    if cb == CPG - 1:
                        nc.vector.tensor_copy(
                            gs[:, (bb - 1) * PITCH : (bb + 1) * PITCH], ps[0:GW, :]
                        )
                    if bb == FB - 1:
                        # hop1: gs block -> DRAM verbatim (50 fat descriptors)
                        h1eng(blk).dma_start(staged[blk][:, :], gs[:, :])
                        # hop2: DRAM -> batch-major flatG rows (2 ops of 8
                        # batches; 296B descriptors)
                        for half in range(FB // H2B):
                            b0 = half * H2B
                            sv = staged[blk][
                                :, b0 * PITCH : (b0 + H2B) * PITCH
                            ].copy()
                            sv.ap = mybir.VecI64Pair(
                                [[PITCH, H2B], [FB * PITCH, GW], [1, PITCH]]
                            )
                            gb0 = blk * FB + b0
                            h2eng(2 * blk + half).dma_start(
                                flatG[gb0 : gb0 + H2B, :], sv
                            )

            # ---------------- Phase B: fixups ----------------
            # flatG row layout per batch: rows i=0:49 are [QtQ(49) | c | QtP(49)
            # | x(49)] at pitch 148; row 49 is [sq(49) | C | sp(49) | dp(49)]
            R49 = (GW - 1) * PITCH

            def dview(col0, stride):
                v = flatG[:, col0 : col0 + 1].copy()
                v.ap = mybir.VecI64Pair([list(v.ap[0])] + [[stride, HW]])
                return v

            dq = dview(0, PITCH + 1)
            sq = flatG[:, R49 : R49 + HW]
            sp = flatG[:, R49 + HW + 1 : R49 + 2 * HW + 1]
            dp = flatG[:, R49 + AC : R49 + AC + HW]

            # batched rsqrt chain over [sq|sp], [dq|dp] (one pass, one
            # ACT round trip instead of two)
            d98 = s_t("d98", 2 * HW)
            s98 = s_t("s98", 2 * HW)
            t98, v98, iv98, n98 = (
                s_t("t98", 2 * HW),
                s_t("v98", 2 * HW),
                s_t("iv98", 2 * HW),
                s_t("n98", 2 * HW),
            )
            inv98 = s_t("inv98", 2 * HW, f16)
            with tc.high_priority():
                nc.vector.tensor_copy(d98[:, 0:HW], dq)
                nc.vector.tensor_copy(d98[:, HW : 2 * HW], dp)
                nc.vector.tensor_copy(s98[:, 0:HW], sq)
                nc.vector.tensor_copy(s98[:, HW : 2 * HW], sp)
                nc.vector.tensor_mul(t98[:], s98[:], s98[:])
                nc.vector.scalar_tensor_tensor(
                    v98[:], t98[:], -1.0 / C, d98[:], Alu.mult, Alu.add
                )
                nc.scalar.activation(t98[:], v98[:], Act.Sqrt)
                nc.vector.reciprocal(iv98[:], t98[:])
                nc.vector.tensor_mul(n98[:], iv98[:], iv98[:])
                nc.vector.tensor_mul(n98[:], n98[:], v98[:])
                nc.vector.tensor_scalar(n98[:], n98[:], -0.5, 1.5, Alu.mult, Alu.add)
                nc.vector.tensor_mul(inv98[:], iv98[:], n98[:])
            inq = inv98[:, 0:HW]
            inp_ = inv98[:, HW : 2 * HW]
            # preload the Exp table now (both Sqrt uses are done) so the
            # load hides under the sim-build DVE ops
            wrm = s_t("wrm", 1)
            nc.vector.memset(wrm[:], 1.0)
            nc.scalar.activation(wrm[:], wrm[:], Act.Exp)

            # sim = (qtp - sq x sp / C) * (inq x inp), in [49,50]-padded fp16
            G3 = flatG[:].rearrange("b (i k) -> b i k", k=PITCH)
            qtp3 = G3[:, 0:HW, PAD : PAD + HW]
            sim3 = simb[:].rearrange("b (q p) -> b q p", p=PAD)
            KS3 = KS[:].rearrange("b (q p) -> b q p", p=PAD)
            tb3 = tb[:].rearrange("b (q p) -> b q p", p=PAD)
            bq = inq.unsqueeze(2).broadcast_to([BS, HW, HW])
            bp = inp_.unsqueeze(1).broadcast_to([BS, HW, HW])
            bsq = sq.unsqueeze(2).broadcast_to([BS, HW, HW])
            bsp = sp.unsqueeze(1).broadcast_to([BS, HW, HW])
            # ssp2 = (-sq/C) x sp on gpsimd (flatG-only deps, starts the
            # moment flatG lands), concurrent with the DVE fixup chain
            s49 = s_t("s49", HW, f16)
            nc.vector.tensor_scalar_mul(s49[:], sq, -1.0 / C)
            bs49 = s49[:].unsqueeze(2).broadcast_to([BS, HW, HW])
            nc.gpsimd.tensor_mul(KS3[:, :, 0:HW], bs49, bsp)  # KS as scratch
            # nrm = inq x inp on DVE (needs the fixups); then centering and
            # the final product, both 2x. High priority: this is the
            # critical path into K-exp and the Sinkhorn.
            with tc.high_priority(offset=1000):
                nc.vector.tensor_mul(tb3[:, :, 0:HW], bq, bp)
                nc.vector.tensor_add(sim3[:, :, 0:HW], qtp3, KS3[:, :, 0:HW])
                nc.vector.tensor_mul(
                    sim3[:, :, 0:HW], sim3[:, :, 0:HW], tb3[:, :, 0:HW]
                )

            # K = exp(sim/eps); Kt via transposed read (ACT)
            nc.scalar.activation(Kb[:], simb[:], Act.Exp, scale=EXPB)
            simT = simb[:].rearrange("b (q p) -> b p q", p=PAD)[:, 0:HW, :]
            KtV = Ktb[:].rearrange("b (p q) -> b p q", q=PAD)[:, :, 0:HW]
            nc.scalar.activation(KtV, simT, Act.Exp, scale=EXPB)

            # w1/w2 reduces on DVE, overlapped with the ACT exps (w2r is
            # the slow strided one -- emitted last so it hides under K-exp)
            w1r, w2r = s_t("w1r"), s_t("w2r")
            w1f = s_t("w1f", HW, f16)
            w2f = s_t("w2f", HW, f16)
            s1s = s_t("s1s", 1)
            nc.vector.tensor_reduce(w1r[:], qtp3, axis=AxX, op=Alu.add)
            G3T = flatG[:].rearrange("b (i k) -> b k i", k=PITCH)
            qtpT = G3T[:, PAD : PAD + HW, 0:HW]  # [b, p, q(stride PITCH)]
            nc.vector.tensor_reduce(
                w2r[:, 0:25], qtpT[:, 0:25, :], axis=AxX, op=Alu.add
            )
            nc.vector.tensor_reduce(
                w2r[:, 25:HW], qtpT[:, 25:HW, :], axis=AxX, op=Alu.add
            )
            for wr, wf in ((w1r, w1f), (w2r, w2f)):
                nc.vector.tensor_scalar(wr[:], wr[:], WSCL / HW, 0.0, Alu.mult, Alu.max)
                nc.vector.tensor_scalar(wr[:], wr[:], 0.001 * WSCL, None, Alu.add)
                nc.vector.tensor_copy(wf[:], wr[:])
            nc.vector.tensor_reduce(s1s[:], w1r[:], axis=AxX, op=Alu.add)

            # KS = K * sim for the final logits (gpsimd, off critical path)
            nc.gpsimd.tensor_mul(KS[:], Kb[:], simb[:])

            # ---------------- Phase B: Sinkhorn ----------------
            K3 = Kb[:].rearrange("b (q p) -> b q p", p=PAD)
            Kt3 = Ktb[:].rearrange("b (p q) -> b p q", q=PAD)
            kv, rkv = s_t("kv"), s_t("rkv")
            bvs = vs[:].unsqueeze(1).broadcast_to([BS, HW, PAD])
            bus = us[:].unsqueeze(1).broadcast_to([BS, HW, PAD])
            for it in range(ITERS):
                # u-step: kv[q] = sum_p K[q,p] v[p]; u = w1 / kv
                if it == 0:
                    nc.vector.tensor_reduce(kv[:], K3, axis=AxX, op=Alu.add)
                else:
                    nc.vector.tensor_mul(tb3, K3, bvs)
                    nc.vector.tensor_reduce(kv[:], tb3, axis=AxX, op=Alu.add)
                nc.vector.reciprocal_approx_fast(rkv[:], kv[:])
                nc.vector.tensor_mul(us[:, 0:HW], w1f[:], rkv[:])
                if HALF and it == ITERS - 1:
                    break
                # v-step: ku[p] = sum_q K[q,p] u[q]; v = w2 / ku
                nc.vector.tensor_mul(tb3, Kt3, bus)
                nc.vector.tensor_reduce(kv[:], tb3, axis=AxX, op=Alu.add)
                nc.vector.reciprocal_approx_fast(rkv[:], kv[:])
                nc.vector.tensor_mul(vs[:, 0:HW], w2f[:], rkv[:])

            # ---------------- Phase B: logits ----------------
            lgr = s_t("lgr", 1)
            lgf = s_t("lgf", 1)
            nc.vector.tensor_mul(tb3, KS3, bvs)  # KS * vs
            nc.vector.tensor_reduce(kv[:], tb3, axis=AxX, op=Alu.add)
            nc.vector.tensor_mul(kv[:], kv[:], w1r[:])
            nc.vector.tensor_mul(kv[:], kv[:], rkv[:])
            nc.vector.tensor_reduce(lgr[:], kv[:], axis=AxX, op=Alu.add)
            nc.vector.reciprocal(rkv[:, 0:1], s1s[:])
            nc.vector.scalar_tensor_tensor(
                lgf[:], lgr[:], TEMP, rkv[:, 0:1], Alu.mult, Alu.mult
            )
            nc.sync.dma_start(outp[:, :], lgf[:])
            if "flatG" in dbgt:
                nc.sync.dma_start(dbgt["flatG"][:, :], flatG[:])
            if "sim" in dbgt:
                nc.sync.dma_start(dbgt["sim"][:, :], simb[:])
            if "K" in dbgt:
                nc.sync.dma_start(dbgt["K"][:, :], Kb[:])
            if "Kt" in dbgt:
                nc.sync.dma_start(dbgt["Kt"][:, :], Ktb[:])

    nc.compile()
    return nc


_NC = None


def _get_nc():
    global _NC
    if _NC is None:
        _NC = build_nc()
    return _NC


def _prep_in_maps(feature_map1, feature_map2):
    q = np.ascontiguousarray(np.asarray(feature_map1, dtype=np.float32)).reshape(
        B_FULL, C, HW
    )
    p = np.ascontiguousarray(np.asarray(feature_map2, dtype=np.float32)).reshape(
        B_FULL, C, HW
    )
    in_maps = []
    for i in range(NCORE):
        sl = slice(i * BS, (i + 1) * BS)
        a32 = np.empty((NGRP, 128, GRP, NCH, AC2), np.float32)
        a32[..., HW] = 1.0
        a32[..., AC2 - 1] = 0.0
        qc = q[sl].reshape(NGRP, GRP, NCH, 128, HW).transpose(0, 3, 1, 2, 4)
        pc = p[sl].reshape(NGRP, GRP, NCH, 128, HW).transpose(0, 3, 1, 2, 4)
        a32[..., 0:HW] = qc
        a32[..., HW + 1 : AC] = pc
        in_maps.append({"aug": a32.astype(np.float16).reshape(NGRP, 128, SLAB)})
    return in_maps


def run(feature_map1, feature_map2, trace=False):
    in_maps = _prep_in_maps(feature_map1, feature_map2)
    nc = _get_nc()
    res = run_bass_kernel_spmd(nc, in_maps, core_ids=list(range(NCORE)), trace=trace)
    out = np.concatenate(
        [np.asarray(res.results[i]["out"]).reshape(BS) for i in range(NCORE)]
    ).astype(np.float32)
    return out, res


def kernel(feature_map1, feature_map2):
    out, _ = run(feature_map1, feature_map2, trace=False)
    return out
